# revision 16
# baseline (speedup 1.0000x reference)
"""ConcatRelationModule Bass kernel for 8 trn2 NeuronCores.

Reference computation (per edge e in [0, 16383)):
    x      = concat(inputs[heads[e], 0, :], inputs[e + 1, 1, :])     # [512]
    h      = tanh(concat(x @ W_FOH, x @ W_FOM) + b1)                 # [1024]
    h2     = tanh(h @ W2 + b2)                                       # [256]
    out[e] = h2 @ W3 + b3                                            # [64]

Strategy: data-parallel over edges (2048 per core, last edge padded).
On-chip layout keeps features on SBUF partitions and edges on the free
dim, so the only transposes are the 4 PE transposes per 128-edge tile
that flip the gathered x from edge-major to feature-major.  Output is
produced as [64, E] per core and transposed back to [E, 64] on host.
"""

import os

import numpy as np
import ml_dtypes

import concourse.bass as bass
import concourse.bacc as bacc
import concourse.mybir as mybir
import concourse.tile as tile
from concourse.bass import IndirectOffsetOnAxis
from concourse.bass_utils import run_bass_kernel_spmd
from concourse.masks import make_identity

N_TOKENS = 16384
LD = 256          # ldims
HID = 512
HID2 = 256
NREL = 64
NCORES = 8
E = N_TOKENS - 1  # 16383 real edges
EPC = N_TOKENS // NCORES  # 2048 edges per core (padded)
P = 128
SUBTILES = EPC // P       # 16 subtiles of 128 edges
GROUP = 512               # edges per matmul free dim
NGROUPS = EPC // GROUP    # 4

# matmul operand dtype ("bf16" or "f32")
RUN_DT = os.environ.get("KERNEL_DT", "bf16")

LAST_RESULTS = None
_CACHE = {}


def _build(dt_str):
    cdt = mybir.dt.bfloat16 if dt_str == "bf16" else mybir.dt.float32
    f32 = mybir.dt.float32

    nc = bacc.Bacc()
    fwd = nc.declare_dram_parameter("fwd", [N_TOKENS, LD], cdt, isOutput=False)
    bwd = nc.declare_dram_parameter("bwd", [EPC, LD], cdt, isOutput=False)
    headsT = nc.declare_dram_parameter(
        "headsT", [P, SUBTILES], mybir.dt.int32, isOutput=False)
    w1 = nc.declare_dram_parameter("w1", [2 * LD, 2 * HID], cdt, isOutput=False)
    w2 = nc.declare_dram_parameter("w2", [2 * HID, HID2], cdt, isOutput=False)
    w3 = nc.declare_dram_parameter("w3", [HID2, NREL], cdt, isOutput=False)
    b1 = nc.declare_dram_parameter("b1", [P, 8], f32, isOutput=False)
    b2 = nc.declare_dram_parameter("b2", [P, 2], f32, isOutput=False)
    b3 = nc.declare_dram_parameter("b3", [NREL, 1], f32, isOutput=False)
    outT = nc.declare_dram_parameter("outT", [NREL, EPC], f32, isOutput=True)

    Tanh = mybir.ActivationFunctionType.Tanh
    Identity = mybir.ActivationFunctionType.Identity

    # variable group sizes: small leading groups so the PE starts after the
    # first gather instead of after four
    GROUPS = [(0, 128), (128, 128), (256, 256), (512, 512), (1024, 512),
              (1536, 512)]

    with tile.TileContext(nc) as tc:
        with (
            tc.tile_pool(name="const", bufs=1) as const_pool,
            tc.tile_pool(name="xg", bufs=6) as xg_pool,
            tc.tile_pool(name="xT", bufs=3) as xT_pool,
            tc.tile_pool(name="h1", bufs=2) as h1_pool,
            tc.tile_pool(name="h2", bufs=2) as h2_pool,
            tc.tile_pool(name="outs", bufs=2) as out_pool,
            tc.tile_pool(name="pt", bufs=3, space="PSUM") as pt_pool,
            tc.tile_pool(name="ph", bufs=2, space="PSUM") as ph_pool,
            tc.tile_pool(name="pj", bufs=2, space="PSUM") as pj_pool,
            tc.tile_pool(name="po", bufs=1, space="PSUM") as po_pool,
        ):
            # head indices first, via SWDGE, so the first gather fires ASAP
            hT_sb = const_pool.tile([P, SUBTILES], mybir.dt.int32)
            nc.gpsimd.dma_start(hT_sb[:], headsT[:])
            ident = const_pool.tile([P, P], cdt)
            make_identity(nc, ident[:])

            xg_tiles = [None] * len(GROUPS)

            def load_group(gi):
                start, size = GROUPS[gi]
                ns = size // P
                # xgg[:, s, 0:256] = gathered head rows for subtile s
                # xgg[:, s, 256:512] = contiguous modifier rows
                xgg = xg_pool.tile([P, ns, 2 * LD], cdt, tag="xgg",
                                   name=f"xgg_{gi}")
                for s in range(ns):
                    t = start // P + s
                    nc.gpsimd.indirect_dma_start(
                        out=xgg[:, s, 0:LD],
                        out_offset=None,
                        in_=fwd[:],
                        in_offset=IndirectOffsetOnAxis(ap=hT_sb[:, t:t + 1], axis=0),
                    )
                nc.sync.dma_start(
                    xgg[:, :, LD:2 * LD],
                    bwd[start:start + size, :].rearrange("(s p) d -> p s d", p=P),
                )
                xg_tiles[gi] = xgg

            load_group(0)
            load_group(1)
            # w1 split per k-chunk so the first L1 matmul only waits on chunk 0
            w1_sb = [const_pool.tile([P, 2 * HID], cdt, tag=f"w1_{kc}",
                                     name=f"w1_{kc}")
                     for kc in range(4)]
            for kc in range(4):
                nc.sync.dma_start(w1_sb[kc][:], w1[kc * P:(kc + 1) * P, :])
            b1_sb = const_pool.tile([P, 8], f32)
            nc.sync.dma_start(b1_sb[:], b1[:])
            load_group(2)
            w2_sb = const_pool.tile([P, 8, HID2], cdt)
            nc.sync.dma_start(w2_sb[:], w2.rearrange("(kc p) j -> p kc j", p=P))
            b2_sb = const_pool.tile([P, 2], f32)
            nc.sync.dma_start(b2_sb[:], b2[:])
            load_group(3)
            w3_sb = const_pool.tile([P, 2, NREL], cdt)
            nc.sync.dma_start(w3_sb[:], w3.rearrange("(kc p) r -> p kc r", p=P))
            b3_sb = const_pool.tile([NREL, 1], f32)
            nc.sync.dma_start(b3_sb[:], b3[:])
            load_group(4)
            load_group(5)

            xT_tiles = [None] * len(GROUPS)

            def emit_transpose(gi):
                start, size = GROUPS[gi]
                xgg = xg_tiles[gi]
                xT = xT_pool.tile([P, 4, size], cdt, tag="xT", name=f"xT_{gi}")
                for kc in range(4):
                    pt = pt_pool.tile([P, size], cdt, tag="pt", name=f"pt_{gi}_{kc}")
                    for s in range(size // P):
                        nc.tensor.transpose(
                            pt[:, s * P:(s + 1) * P],
                            xgg[:, s, kc * P:(kc + 1) * P], ident[:])
                    nc.vector.tensor_copy(out=xT[:, kc, :], in_=pt[:])
                xT_tiles[gi] = xT

            emit_transpose(0)
            for gi, (start, size) in enumerate(GROUPS):
                xT = xT_tiles[gi]
                # ---- layer 1: h = tanh(W1.T-chunks @ x + b1), 8 h-chunks ----
                h1 = h1_pool.tile([P, 8, size], cdt, tag="h1", name=f"h1_{gi}")
                for hc in range(8):
                    ph = ph_pool.tile([P, size], f32, tag="ph",
                                      name=f"ph_{gi}_{hc}")
                    for kc in range(4):
                        nc.tensor.matmul(
                            out=ph[:],
                            lhsT=w1_sb[kc][:, hc * P:(hc + 1) * P],
                            rhs=xT[:, kc, :],
                            start=(kc == 0),
                            stop=(kc == 3),
                        )
                    nc.scalar.activation(
                        out=h1[:, hc, :], in_=ph[:], func=Tanh,
                        bias=b1_sb[:, hc:hc + 1],
                    )

                # transpose the NEXT group now so its DVE copies finish
                # while this group's L2/L3 run on the PE
                if gi + 1 < len(GROUPS):
                    emit_transpose(gi + 1)

                # ---- layer 2: h2 = tanh(W2-chunks @ h + b2), 2 j-chunks ----
                h2 = h2_pool.tile([P, 2, size], cdt, tag="h2", name=f"h2_{gi}")
                for jc in range(2):
                    pj = pj_pool.tile([P, size], f32, tag="pj",
                                      name=f"pj_{gi}_{jc}")
                    for kc in range(8):
                        nc.tensor.matmul(
                            out=pj[:],
                            lhsT=w2_sb[:, kc, jc * P:(jc + 1) * P],
                            rhs=h1[:, kc, :],
                            start=(kc == 0),
                            stop=(kc == 7),
                        )
                    nc.scalar.activation(
                        out=h2[:, jc, :], in_=pj[:], func=Tanh,
                        bias=b2_sb[:, jc:jc + 1],
                    )

                # ---- layer 3: out = W3-chunks @ h2 + b3 ----
                po = po_pool.tile([NREL, size], f32, tag="po", name=f"po_{gi}")
                for kc in range(2):
                    nc.tensor.matmul(
                        out=po[:],
                        lhsT=w3_sb[:, kc, :],
                        rhs=h2[:, kc, :],
                        start=(kc == 0),
                        stop=(kc == 1),
                    )
                o = out_pool.tile([NREL, size], f32, tag="o", name=f"o_{gi}")
                nc.scalar.activation(
                    out=o[:], in_=po[:], func=Identity, bias=b3_sb[:, 0:1]
                )
                nc.sync.dma_start(outT[:, start:start + size], o[:])

    nc.finalize()
    return nc


def kernel(inputs, rhidLayerFOH, rhidLayerFOM, rcatBias, rhid2Layer, rhid2Bias,
           routLayer, routBias, heads):
    global LAST_RESULTS

    inputs = np.asarray(inputs, dtype=np.float32)
    heads = np.asarray(heads)

    if RUN_DT == "bf16":
        wdt = ml_dtypes.bfloat16
    else:
        wdt = np.float32

    fwd = np.ascontiguousarray(inputs[:, 0, :]).astype(wdt)      # [N, 256]
    bwd_full = inputs[:, 1, :]                                   # [N, 256]
    # mods for edge e is e+1; pad edge 16383 with mod 16383 (garbage, dropped)
    mods_pad = np.concatenate([np.arange(1, N_TOKENS), [N_TOKENS - 1]]).astype(np.int64)
    heads_pad = np.concatenate([heads.astype(np.int64), [0]]).astype(np.int32)

    w1 = np.ascontiguousarray(
        np.concatenate([np.asarray(rhidLayerFOH), np.asarray(rhidLayerFOM)], axis=1)
    ).astype(wdt)                                                # [512, 1024]
    w2 = np.ascontiguousarray(np.asarray(rhid2Layer)).astype(wdt)  # [1024, 256]
    w3 = np.ascontiguousarray(np.asarray(routLayer)).astype(wdt)   # [256, 64]
    b1 = np.ascontiguousarray(
        np.asarray(rcatBias, dtype=np.float32).reshape(8, P).T)    # [128, 8]
    b2 = np.ascontiguousarray(
        np.asarray(rhid2Bias, dtype=np.float32).reshape(2, P).T)   # [128, 2]
    b3 = np.ascontiguousarray(
        np.asarray(routBias, dtype=np.float32).reshape(1, NREL).T)  # [64, 1]

    in_maps = []
    for c in range(NCORES):
        sl = slice(c * EPC, (c + 1) * EPC)
        bwd_c = np.ascontiguousarray(bwd_full[mods_pad[sl]]).astype(wdt)  # [2048, 256]
        headsT_c = np.ascontiguousarray(
            heads_pad[sl].reshape(SUBTILES, P).T)                 # [128, 16]
        in_maps.append({
            "fwd": fwd, "bwd": bwd_c, "headsT": headsT_c,
            "w1": w1, "w2": w2, "w3": w3, "b1": b1, "b2": b2, "b3": b3,
        })

    if RUN_DT not in _CACHE:
        _CACHE[RUN_DT] = _build(RUN_DT)
    nc = _CACHE[RUN_DT]

    trace_dir = os.environ.get("KERNEL_TRACE_DIR") or None
    res = run_bass_kernel_spmd(nc, in_maps, list(range(NCORES)), tmpdir=trace_dir)
    LAST_RESULTS = res

    outT = np.concatenate([r["outT"] for r in res.results], axis=1)  # [64, 16384]
    return np.ascontiguousarray(outT.T[:E]).astype(np.float32)       # [16383, 64]


# revision 18
# speedup vs baseline: 1.0629x; 1.0629x over previous
"""ConcatRelationModule Bass kernel for 8 trn2 NeuronCores.

Reference computation (per edge e in [0, 16383)):
    x      = concat(inputs[heads[e], 0, :], inputs[e + 1, 1, :])     # [512]
    h      = tanh(concat(x @ W_FOH, x @ W_FOM) + b1)                 # [1024]
    h2     = tanh(h @ W2 + b2)                                       # [256]
    out[e] = h2 @ W3 + b3                                            # [64]

Strategy: data-parallel over edges (2048 per core, last edge padded).
On-chip layout keeps features on SBUF partitions and edges on the free
dim, so the only transposes are the 4 PE transposes per 128-edge tile
that flip the gathered x from edge-major to feature-major.  Output is
produced as [64, E] per core and transposed back to [E, 64] on host.
"""

import os

import numpy as np
import ml_dtypes

import concourse.bass as bass
import concourse.bacc as bacc
import concourse.mybir as mybir
import concourse.tile as tile
from concourse.bass import IndirectOffsetOnAxis
from concourse.bass_utils import run_bass_kernel_spmd
from concourse.masks import make_identity

N_TOKENS = 16384
LD = 256          # ldims
HID = 512
HID2 = 256
NREL = 64
NCORES = 8
E = N_TOKENS - 1  # 16383 real edges
EPC = N_TOKENS // NCORES  # 2048 edges per core (padded)
P = 128
SUBTILES = EPC // P       # 16 subtiles of 128 edges
GROUP = 512               # edges per matmul free dim
NGROUPS = EPC // GROUP    # 4

# matmul operand dtype ("bf16" or "f32")
RUN_DT = os.environ.get("KERNEL_DT", "bf16")

LAST_RESULTS = None
_CACHE = {}


def _build(dt_str):
    cdt = mybir.dt.bfloat16 if dt_str == "bf16" else mybir.dt.float32
    f32 = mybir.dt.float32

    nc = bacc.Bacc()
    fwd = nc.declare_dram_parameter("fwd", [N_TOKENS, LD], cdt, isOutput=False)
    bwd = nc.declare_dram_parameter("bwd", [EPC, LD], cdt, isOutput=False)
    headsT = nc.declare_dram_parameter(
        "headsT", [P, SUBTILES], mybir.dt.int32, isOutput=False)
    w1 = nc.declare_dram_parameter("w1", [2 * LD, 2 * HID], cdt, isOutput=False)
    w2 = nc.declare_dram_parameter("w2", [2 * HID, HID2], cdt, isOutput=False)
    w3 = nc.declare_dram_parameter("w3", [HID2, NREL], cdt, isOutput=False)
    b1 = nc.declare_dram_parameter("b1", [P, 8], f32, isOutput=False)
    b2 = nc.declare_dram_parameter("b2", [P, 2], f32, isOutput=False)
    b3 = nc.declare_dram_parameter("b3", [NREL, 1], f32, isOutput=False)
    outT = nc.declare_dram_parameter("outT", [NREL, EPC], f32, isOutput=True)

    Tanh = mybir.ActivationFunctionType.Tanh
    Identity = mybir.ActivationFunctionType.Identity

    # variable group sizes: small leading groups so the PE starts after the
    # first gather instead of after four
    GROUPS = [(0, 128), (128, 128), (256, 256), (512, 512), (1024, 512),
              (1536, 512)]

    with tile.TileContext(nc) as tc:
        with (
            tc.tile_pool(name="const", bufs=1) as const_pool,
            tc.tile_pool(name="xg", bufs=6) as xg_pool,
            tc.tile_pool(name="xT", bufs=3) as xT_pool,
            tc.tile_pool(name="h1", bufs=2) as h1_pool,
            tc.tile_pool(name="h2", bufs=2) as h2_pool,
            tc.tile_pool(name="outs", bufs=2) as out_pool,
            tc.tile_pool(name="pt", bufs=2, space="PSUM") as pt_pool,
            tc.tile_pool(name="ph", bufs=3, space="PSUM") as ph_pool,
            tc.tile_pool(name="pj", bufs=2, space="PSUM") as pj_pool,
            tc.tile_pool(name="po", bufs=1, space="PSUM") as po_pool,
        ):
            # head indices first on the HWDGE queue so the first gather
            # fires as soon as its ~3us descriptor latency allows
            hT_sb = const_pool.tile([P, SUBTILES], mybir.dt.int32)
            nc.sync.dma_start(hT_sb[:], headsT[:])
            ident = const_pool.tile([P, P], cdt)
            make_identity(nc, ident[:])

            xg_tiles = [None] * len(GROUPS)

            def load_group(gi):
                start, size = GROUPS[gi]
                ns = size // P
                # xgg[:, s, 0:256] = gathered head rows for subtile s
                # xgg[:, s, 256:512] = contiguous modifier rows
                xgg = xg_pool.tile([P, ns, 2 * LD], cdt, tag="xgg",
                                   name=f"xgg_{gi}")
                for s in range(ns):
                    t = start // P + s
                    nc.gpsimd.indirect_dma_start(
                        out=xgg[:, s, 0:LD],
                        out_offset=None,
                        in_=fwd[:],
                        in_offset=IndirectOffsetOnAxis(ap=hT_sb[:, t:t + 1], axis=0),
                    )
                nc.sync.dma_start(
                    xgg[:, :, LD:2 * LD],
                    bwd[start:start + size, :].rearrange("(s p) d -> p s d", p=P),
                )
                xg_tiles[gi] = xgg

            load_group(0)
            load_group(1)
            # w1 split per k-chunk so the first L1 matmul only waits on chunk 0
            w1_sb = [const_pool.tile([P, 2 * HID], cdt, tag=f"w1_{kc}",
                                     name=f"w1_{kc}")
                     for kc in range(4)]
            for kc in range(4):
                nc.sync.dma_start(w1_sb[kc][:], w1[kc * P:(kc + 1) * P, :])
            b1_sb = const_pool.tile([P, 8], f32)
            nc.sync.dma_start(b1_sb[:], b1[:])
            load_group(2)
            w2_sb = const_pool.tile([P, 8, HID2], cdt)
            nc.sync.dma_start(w2_sb[:], w2.rearrange("(kc p) j -> p kc j", p=P))
            b2_sb = const_pool.tile([P, 2], f32)
            nc.sync.dma_start(b2_sb[:], b2[:])
            load_group(3)
            w3_sb = const_pool.tile([P, 2, NREL], cdt)
            nc.sync.dma_start(w3_sb[:], w3.rearrange("(kc p) r -> p kc r", p=P))
            b3_sb = const_pool.tile([NREL, 1], f32)
            nc.sync.dma_start(b3_sb[:], b3[:])
            load_group(4)
            load_group(5)

            xT_tiles = [None] * len(GROUPS)

            def emit_transpose(gi):
                start, size = GROUPS[gi]
                xgg = xg_tiles[gi]
                xT = xT_pool.tile([P, 4, size], cdt, tag="xT", name=f"xT_{gi}")
                for kc in range(4):
                    pt = pt_pool.tile([P, size], cdt, tag="pt", name=f"pt_{gi}_{kc}")
                    for s in range(size // P):
                        nc.tensor.transpose(
                            pt[:, s * P:(s + 1) * P],
                            xgg[:, s, kc * P:(kc + 1) * P], ident[:])
                    nc.vector.tensor_copy(out=xT[:, kc, :], in_=pt[:])
                xT_tiles[gi] = xT

            emit_transpose(0)
            for gi, (start, size) in enumerate(GROUPS):
                xT = xT_tiles[gi]
                # ---- layer 1: h = tanh(W1.T-chunks @ x + b1), 8 h-chunks ----
                h1 = h1_pool.tile([P, 8, size], cdt, tag="h1", name=f"h1_{gi}")
                for hc in range(8):
                    ph = ph_pool.tile([P, size], f32, tag="ph",
                                      name=f"ph_{gi}_{hc}")
                    for kc in range(4):
                        nc.tensor.matmul(
                            out=ph[:],
                            lhsT=w1_sb[kc][:, hc * P:(hc + 1) * P],
                            rhs=xT[:, kc, :],
                            start=(kc == 0),
                            stop=(kc == 3),
                        )
                    nc.scalar.activation(
                        out=h1[:, hc, :], in_=ph[:], func=Tanh,
                        bias=b1_sb[:, hc:hc + 1],
                    )

                # transpose the NEXT group now so its DVE copies finish
                # while this group's L2/L3 run on the PE
                if gi + 1 < len(GROUPS):
                    emit_transpose(gi + 1)

                # ---- layer 2: h2 = tanh(W2-chunks @ h + b2), 2 j-chunks ----
                h2 = h2_pool.tile([P, 2, size], cdt, tag="h2", name=f"h2_{gi}")
                for jc in range(2):
                    pj = pj_pool.tile([P, size], f32, tag="pj",
                                      name=f"pj_{gi}_{jc}")
                    for kc in range(8):
                        nc.tensor.matmul(
                            out=pj[:],
                            lhsT=w2_sb[:, kc, jc * P:(jc + 1) * P],
                            rhs=h1[:, kc, :],
                            start=(kc == 0),
                            stop=(kc == 7),
                        )
                    nc.scalar.activation(
                        out=h2[:, jc, :], in_=pj[:], func=Tanh,
                        bias=b2_sb[:, jc:jc + 1],
                    )

                # ---- layer 3: out = W3-chunks @ h2 + b3 ----
                po = po_pool.tile([NREL, size], f32, tag="po", name=f"po_{gi}")
                for kc in range(2):
                    nc.tensor.matmul(
                        out=po[:],
                        lhsT=w3_sb[:, kc, :],
                        rhs=h2[:, kc, :],
                        start=(kc == 0),
                        stop=(kc == 1),
                    )
                o = out_pool.tile([NREL, size], f32, tag="o", name=f"o_{gi}")
                nc.scalar.activation(
                    out=o[:], in_=po[:], func=Identity, bias=b3_sb[:, 0:1]
                )
                nc.sync.dma_start(outT[:, start:start + size], o[:])

    nc.finalize()
    return nc


def kernel(inputs, rhidLayerFOH, rhidLayerFOM, rcatBias, rhid2Layer, rhid2Bias,
           routLayer, routBias, heads):
    global LAST_RESULTS

    inputs = np.asarray(inputs, dtype=np.float32)
    heads = np.asarray(heads)

    if RUN_DT == "bf16":
        wdt = ml_dtypes.bfloat16
    else:
        wdt = np.float32

    fwd = np.ascontiguousarray(inputs[:, 0, :]).astype(wdt)      # [N, 256]
    bwd_full = inputs[:, 1, :]                                   # [N, 256]
    # mods for edge e is e+1; pad edge 16383 with mod 16383 (garbage, dropped)
    mods_pad = np.concatenate([np.arange(1, N_TOKENS), [N_TOKENS - 1]]).astype(np.int64)
    heads_pad = np.concatenate([heads.astype(np.int64), [0]]).astype(np.int32)

    w1 = np.ascontiguousarray(
        np.concatenate([np.asarray(rhidLayerFOH), np.asarray(rhidLayerFOM)], axis=1)
    ).astype(wdt)                                                # [512, 1024]
    w2 = np.ascontiguousarray(np.asarray(rhid2Layer)).astype(wdt)  # [1024, 256]
    w3 = np.ascontiguousarray(np.asarray(routLayer)).astype(wdt)   # [256, 64]
    b1 = np.ascontiguousarray(
        np.asarray(rcatBias, dtype=np.float32).reshape(8, P).T)    # [128, 8]
    b2 = np.ascontiguousarray(
        np.asarray(rhid2Bias, dtype=np.float32).reshape(2, P).T)   # [128, 2]
    b3 = np.ascontiguousarray(
        np.asarray(routBias, dtype=np.float32).reshape(1, NREL).T)  # [64, 1]

    in_maps = []
    for c in range(NCORES):
        sl = slice(c * EPC, (c + 1) * EPC)
        bwd_c = np.ascontiguousarray(bwd_full[mods_pad[sl]]).astype(wdt)  # [2048, 256]
        headsT_c = np.ascontiguousarray(
            heads_pad[sl].reshape(SUBTILES, P).T)                 # [128, 16]
        in_maps.append({
            "fwd": fwd, "bwd": bwd_c, "headsT": headsT_c,
            "w1": w1, "w2": w2, "w3": w3, "b1": b1, "b2": b2, "b3": b3,
        })

    if RUN_DT not in _CACHE:
        _CACHE[RUN_DT] = _build(RUN_DT)
    nc = _CACHE[RUN_DT]

    trace_dir = os.environ.get("KERNEL_TRACE_DIR") or None
    res = run_bass_kernel_spmd(nc, in_maps, list(range(NCORES)), tmpdir=trace_dir)
    LAST_RESULTS = res

    outT = np.concatenate([r["outT"] for r in res.results], axis=1)  # [64, 16384]
    return np.ascontiguousarray(outT.T[:E]).astype(np.float32)       # [16383, 64]


# revision 22
# speedup vs baseline: 1.0651x; 1.0020x over previous
"""ConcatRelationModule Bass kernel for 8 trn2 NeuronCores.

Reference computation (per edge e in [0, 16383)):
    x      = concat(inputs[heads[e], 0, :], inputs[e + 1, 1, :])     # [512]
    h      = tanh(concat(x @ W_FOH, x @ W_FOM) + b1)                 # [1024]
    h2     = tanh(h @ W2 + b2)                                       # [256]
    out[e] = h2 @ W3 + b3                                            # [64]

Strategy: data-parallel over edges (2048 per core, last edge padded).
On-chip layout keeps features on SBUF partitions and edges on the free
dim, so the only transposes are the 4 PE transposes per 128-edge tile
that flip the gathered x from edge-major to feature-major.  Output is
produced as [64, E] per core and transposed back to [E, 64] on host.
"""

import os

import numpy as np
import ml_dtypes

import concourse.bass as bass
import concourse.bacc as bacc
import concourse.mybir as mybir
import concourse.tile as tile
from concourse.bass import IndirectOffsetOnAxis
from concourse.bass_utils import run_bass_kernel_spmd
from concourse.masks import make_identity

N_TOKENS = 16384
LD = 256          # ldims
HID = 512
HID2 = 256
NREL = 64
NCORES = 8
E = N_TOKENS - 1  # 16383 real edges
EPC = N_TOKENS // NCORES  # 2048 edges per core (padded)
P = 128
SUBTILES = EPC // P       # 16 subtiles of 128 edges
GROUP = 512               # edges per matmul free dim
NGROUPS = EPC // GROUP    # 4

# matmul operand dtype ("bf16" or "f32")
RUN_DT = os.environ.get("KERNEL_DT", "bf16")

LAST_RESULTS = None
_CACHE = {}


def _build(dt_str):
    cdt = mybir.dt.bfloat16 if dt_str == "bf16" else mybir.dt.float32
    f32 = mybir.dt.float32

    nc = bacc.Bacc()
    fwd = nc.declare_dram_parameter("fwd", [N_TOKENS, LD], cdt, isOutput=False)
    bwd = nc.declare_dram_parameter("bwd", [EPC, LD], cdt, isOutput=False)
    headsT = nc.declare_dram_parameter(
        "headsT", [P, SUBTILES], mybir.dt.int32, isOutput=False)
    w1 = nc.declare_dram_parameter("w1", [2 * LD, 2 * HID], cdt, isOutput=False)
    w2 = nc.declare_dram_parameter("w2", [2 * HID, HID2], cdt, isOutput=False)
    w3 = nc.declare_dram_parameter("w3", [HID2, NREL], cdt, isOutput=False)
    b1 = nc.declare_dram_parameter("b1", [P, 8], f32, isOutput=False)
    b2 = nc.declare_dram_parameter("b2", [P, 2], f32, isOutput=False)
    b3 = nc.declare_dram_parameter("b3", [NREL, 1], f32, isOutput=False)
    outT = nc.declare_dram_parameter("outT", [NREL, EPC], f32, isOutput=True)

    Tanh = mybir.ActivationFunctionType.Tanh
    Identity = mybir.ActivationFunctionType.Identity

    # variable group sizes: small leading groups so the PE starts after the
    # first gather instead of after four
    GROUPS = [(0, 128), (128, 128), (256, 256), (512, 512), (1024, 512),
              (1536, 512)]

    with tile.TileContext(nc) as tc:
        with (
            tc.tile_pool(name="const", bufs=1) as const_pool,
            tc.tile_pool(name="xh", bufs=6) as xh_pool,
            tc.tile_pool(name="xm", bufs=6) as xm_pool,
            tc.tile_pool(name="xT", bufs=8) as xT_pool,
            tc.tile_pool(name="h1", bufs=16) as h1_pool,
            tc.tile_pool(name="h2", bufs=4) as h2_pool,
            tc.tile_pool(name="outs", bufs=2) as out_pool,
            tc.tile_pool(name="pt", bufs=2, space="PSUM") as pt_pool,
            tc.tile_pool(name="ph", bufs=3, space="PSUM") as ph_pool,
            tc.tile_pool(name="pj", bufs=2, space="PSUM") as pj_pool,
            tc.tile_pool(name="po", bufs=1, space="PSUM") as po_pool,
        ):
            # head indices first on the HWDGE queue so the first gather
            # fires as soon as its ~3us descriptor latency allows
            hT_sb = const_pool.tile([P, SUBTILES], mybir.dt.int32)
            nc.sync.dma_start(hT_sb[:], headsT[:])
            ident = const_pool.tile([P, P], cdt)
            make_identity(nc, ident[:])

            xg_tiles = [None] * len(GROUPS)

            def load_group(gi):
                start, size = GROUPS[gi]
                ns = size // P
                # xh: gathered head rows; xm: contiguous modifier rows.
                # Separate tiles so transposes of the gather half never wait
                # on the bwd DMA (tile-granular dependency tracking).
                xh = xh_pool.tile([P, ns, LD], cdt, tag="xh", name=f"xh_{gi}")
                xm = xm_pool.tile([P, ns, LD], cdt, tag="xm", name=f"xm_{gi}")
                for s in range(ns):
                    t = start // P + s
                    nc.gpsimd.indirect_dma_start(
                        out=xh[:, s, :],
                        out_offset=None,
                        in_=fwd[:],
                        in_offset=IndirectOffsetOnAxis(ap=hT_sb[:, t:t + 1], axis=0),
                    )
                nc.sync.dma_start(
                    xm[:],
                    bwd[start:start + size, :].rearrange("(s p) d -> p s d", p=P),
                )
                xg_tiles[gi] = (xh, xm)

            load_group(0)
            load_group(1)
            # w1 split per k-chunk so the first L1 matmul only waits on chunk 0
            w1_sb = [const_pool.tile([P, 2 * HID], cdt, tag=f"w1_{kc}",
                                     name=f"w1_{kc}")
                     for kc in range(4)]
            for kc in range(4):
                nc.sync.dma_start(w1_sb[kc][:], w1[kc * P:(kc + 1) * P, :])
            b1_sb = const_pool.tile([P, 8], f32)
            nc.sync.dma_start(b1_sb[:], b1[:])
            load_group(2)
            w2_sb = const_pool.tile([P, 8, HID2], cdt)
            nc.sync.dma_start(w2_sb[:], w2.rearrange("(kc p) j -> p kc j", p=P))
            b2_sb = const_pool.tile([P, 2], f32)
            nc.sync.dma_start(b2_sb[:], b2[:])
            load_group(3)
            w3_sb = const_pool.tile([P, 2, NREL], cdt)
            nc.sync.dma_start(w3_sb[:], w3.rearrange("(kc p) r -> p kc r", p=P))
            b3_sb = const_pool.tile([NREL, 1], f32)
            nc.sync.dma_start(b3_sb[:], b3[:])
            load_group(4)
            load_group(5)

            xT_tiles = [None] * len(GROUPS)

            def emit_transpose(gi):
                start, size = GROUPS[gi]
                xh, xm = xg_tiles[gi]
                xTs = []
                for kc in range(4):
                    src = xh if kc < 2 else xm
                    col = (kc % 2) * P
                    pt = pt_pool.tile([P, size], cdt, tag="pt", name=f"pt_{gi}_{kc}")
                    for s in range(size // P):
                        nc.tensor.transpose(
                            pt[:, s * P:(s + 1) * P],
                            src[:, s, col:col + P], ident[:])
                    xT = xT_pool.tile([P, size], cdt, tag="xT",
                                      name=f"xT_{gi}_{kc}")
                    nc.vector.tensor_copy(out=xT[:], in_=pt[:])
                    xTs.append(xT)
                xT_tiles[gi] = xTs

            emit_transpose(0)
            for gi, (start, size) in enumerate(GROUPS):
                xTs = xT_tiles[gi]
                # ---- layer 1: h = tanh(W1.T-chunks @ x + b1), 8 h-chunks ----
                h1s = []
                for hc in range(8):
                    ph = ph_pool.tile([P, size], f32, tag="ph",
                                      name=f"ph_{gi}_{hc}")
                    for kc in range(4):
                        nc.tensor.matmul(
                            out=ph[:],
                            lhsT=w1_sb[kc][:, hc * P:(hc + 1) * P],
                            rhs=xTs[kc][:],
                            start=(kc == 0),
                            stop=(kc == 3),
                        )
                    h1 = h1_pool.tile([P, size], cdt, tag="h1",
                                      name=f"h1_{gi}_{hc}")
                    nc.scalar.activation(
                        out=h1[:], in_=ph[:], func=Tanh,
                        bias=b1_sb[:, hc:hc + 1],
                    )
                    h1s.append(h1)

                # transpose the NEXT group now so its DVE copies finish
                # while this group's L2/L3 run on the PE
                if gi + 1 < len(GROUPS):
                    emit_transpose(gi + 1)

                # ---- layer 2: h2 = tanh(W2-chunks @ h + b2), 2 j-chunks ----
                h2s = []
                for jc in range(2):
                    pj = pj_pool.tile([P, size], f32, tag="pj",
                                      name=f"pj_{gi}_{jc}")
                    for kc in range(8):
                        nc.tensor.matmul(
                            out=pj[:],
                            lhsT=w2_sb[:, kc, jc * P:(jc + 1) * P],
                            rhs=h1s[kc][:],
                            start=(kc == 0),
                            stop=(kc == 7),
                        )
                    h2 = h2_pool.tile([P, size], cdt, tag="h2",
                                      name=f"h2_{gi}_{jc}")
                    nc.scalar.activation(
                        out=h2[:], in_=pj[:], func=Tanh,
                        bias=b2_sb[:, jc:jc + 1],
                    )
                    h2s.append(h2)

                # ---- layer 3: out = W3-chunks @ h2 + b3 ----
                po = po_pool.tile([NREL, size], f32, tag="po", name=f"po_{gi}")
                for kc in range(2):
                    nc.tensor.matmul(
                        out=po[:],
                        lhsT=w3_sb[:, kc, :],
                        rhs=h2s[kc][:],
                        start=(kc == 0),
                        stop=(kc == 1),
                    )
                o = out_pool.tile([NREL, size], f32, tag="o", name=f"o_{gi}")
                nc.scalar.activation(
                    out=o[:], in_=po[:], func=Identity, bias=b3_sb[:, 0:1]
                )
                nc.sync.dma_start(outT[:, start:start + size], o[:])

    nc.finalize()
    return nc


def kernel(inputs, rhidLayerFOH, rhidLayerFOM, rcatBias, rhid2Layer, rhid2Bias,
           routLayer, routBias, heads):
    global LAST_RESULTS

    inputs = np.asarray(inputs, dtype=np.float32)
    heads = np.asarray(heads)

    if RUN_DT == "bf16":
        wdt = ml_dtypes.bfloat16
    else:
        wdt = np.float32

    fwd = np.ascontiguousarray(inputs[:, 0, :]).astype(wdt)      # [N, 256]
    bwd_full = inputs[:, 1, :]                                   # [N, 256]
    # mods for edge e is e+1; pad edge 16383 with mod 16383 (garbage, dropped)
    mods_pad = np.concatenate([np.arange(1, N_TOKENS), [N_TOKENS - 1]]).astype(np.int64)
    heads_pad = np.concatenate([heads.astype(np.int64), [0]]).astype(np.int32)

    w1 = np.ascontiguousarray(
        np.concatenate([np.asarray(rhidLayerFOH), np.asarray(rhidLayerFOM)], axis=1)
    ).astype(wdt)                                                # [512, 1024]
    w2 = np.ascontiguousarray(np.asarray(rhid2Layer)).astype(wdt)  # [1024, 256]
    w3 = np.ascontiguousarray(np.asarray(routLayer)).astype(wdt)   # [256, 64]
    b1 = np.ascontiguousarray(
        np.asarray(rcatBias, dtype=np.float32).reshape(8, P).T)    # [128, 8]
    b2 = np.ascontiguousarray(
        np.asarray(rhid2Bias, dtype=np.float32).reshape(2, P).T)   # [128, 2]
    b3 = np.ascontiguousarray(
        np.asarray(routBias, dtype=np.float32).reshape(1, NREL).T)  # [64, 1]

    in_maps = []
    for c in range(NCORES):
        sl = slice(c * EPC, (c + 1) * EPC)
        bwd_c = np.ascontiguousarray(bwd_full[mods_pad[sl]]).astype(wdt)  # [2048, 256]
        headsT_c = np.ascontiguousarray(
            heads_pad[sl].reshape(SUBTILES, P).T)                 # [128, 16]
        in_maps.append({
            "fwd": fwd, "bwd": bwd_c, "headsT": headsT_c,
            "w1": w1, "w2": w2, "w3": w3, "b1": b1, "b2": b2, "b3": b3,
        })

    if RUN_DT not in _CACHE:
        _CACHE[RUN_DT] = _build(RUN_DT)
    nc = _CACHE[RUN_DT]

    trace_dir = os.environ.get("KERNEL_TRACE_DIR") or None
    res = run_bass_kernel_spmd(nc, in_maps, list(range(NCORES)), tmpdir=trace_dir)
    LAST_RESULTS = res

    outT = np.concatenate([r["outT"] for r in res.results], axis=1)  # [64, 16384]
    return np.ascontiguousarray(outT.T[:E]).astype(np.float32)       # [16383, 64]


# revision 25
# speedup vs baseline: 1.1083x; 1.0405x over previous
"""ConcatRelationModule Bass kernel for 8 trn2 NeuronCores.

Reference computation (per edge e in [0, 16383)):
    x      = concat(inputs[heads[e], 0, :], inputs[e + 1, 1, :])     # [512]
    h      = tanh(concat(x @ W_FOH, x @ W_FOM) + b1)                 # [1024]
    h2     = tanh(h @ W2 + b2)                                       # [256]
    out[e] = h2 @ W3 + b3                                            # [64]

Strategy: data-parallel over edges (2048 per core, last edge padded).
On-chip layout keeps features on SBUF partitions and edges on the free
dim, so the only transposes are the 4 PE transposes per 128-edge tile
that flip the gathered x from edge-major to feature-major.  Output is
produced as [64, E] per core and transposed back to [E, 64] on host.
"""

import os

import numpy as np
import ml_dtypes

import concourse.bass as bass
import concourse.bacc as bacc
import concourse.mybir as mybir
import concourse.tile as tile
from concourse.bass import IndirectOffsetOnAxis
from concourse.bass_utils import run_bass_kernel_spmd
from concourse.masks import make_identity

N_TOKENS = 16384
LD = 256          # ldims
HID = 512
HID2 = 256
NREL = 64
NCORES = 8
E = N_TOKENS - 1  # 16383 real edges
EPC = N_TOKENS // NCORES  # 2048 edges per core (padded)
P = 128
SUBTILES = EPC // P       # 16 subtiles of 128 edges
GROUP = 512               # edges per matmul free dim
NGROUPS = EPC // GROUP    # 4

# matmul operand dtype ("bf16" or "f32")
RUN_DT = os.environ.get("KERNEL_DT", "bf16")

LAST_RESULTS = None
_CACHE = {}


def _build(dt_str):
    cdt = mybir.dt.bfloat16 if dt_str == "bf16" else mybir.dt.float32
    f32 = mybir.dt.float32

    nc = bacc.Bacc()
    fwd = nc.declare_dram_parameter("fwd", [N_TOKENS, LD], cdt, isOutput=False)
    bwd = nc.declare_dram_parameter("bwd", [EPC, LD], cdt, isOutput=False)
    headsT = nc.declare_dram_parameter(
        "headsT", [P, SUBTILES], mybir.dt.int32, isOutput=False)
    w1 = nc.declare_dram_parameter("w1", [2 * LD, 2 * HID], cdt, isOutput=False)
    w2 = nc.declare_dram_parameter("w2", [2 * HID, HID2], cdt, isOutput=False)
    w3 = nc.declare_dram_parameter("w3", [HID2, NREL], cdt, isOutput=False)
    b1 = nc.declare_dram_parameter("b1", [P, 8], f32, isOutput=False)
    b2 = nc.declare_dram_parameter("b2", [P, 2], f32, isOutput=False)
    b3 = nc.declare_dram_parameter("b3", [NREL, 1], f32, isOutput=False)
    outT = nc.declare_dram_parameter("outT", [NREL, EPC], f32, isOutput=True)

    Tanh = mybir.ActivationFunctionType.Tanh
    Identity = mybir.ActivationFunctionType.Identity

    # variable group sizes: small leading groups so the PE starts after the
    # first gather instead of after four; small last group to shorten the
    # final ACT + out-DMA chain before the end-of-kernel barrier
    GROUPS = [(0, 128), (128, 128), (256, 512), (768, 512), (1280, 512),
              (1792, 256)]

    with tile.TileContext(nc) as tc:
        with (
            tc.tile_pool(name="const", bufs=1) as const_pool,
            tc.tile_pool(name="xh", bufs=6) as xh_pool,
            tc.tile_pool(name="xm", bufs=6) as xm_pool,
            tc.tile_pool(name="xT", bufs=8) as xT_pool,
            tc.tile_pool(name="h1", bufs=16) as h1_pool,
            tc.tile_pool(name="h2", bufs=4) as h2_pool,
            tc.tile_pool(name="outs", bufs=2) as out_pool,
            tc.tile_pool(name="pt", bufs=3, space="PSUM") as pt_pool,
            tc.tile_pool(name="ph", bufs=3, space="PSUM") as ph_pool,
            tc.tile_pool(name="pj", bufs=2, space="PSUM") as pj_pool,
        ):
            po_pool = pt_pool  # transposes and L3 share 3 psum banks
            # head indices first on the HWDGE queue so the first gather
            # fires as soon as its ~3us descriptor latency allows
            hT_sb = const_pool.tile([P, SUBTILES], mybir.dt.int32)
            nc.sync.dma_start(hT_sb[:], headsT[:])
            ident = const_pool.tile([P, P], cdt)
            make_identity(nc, ident[:])

            xg_tiles = [None] * len(GROUPS)

            def load_group(gi):
                start, size = GROUPS[gi]
                ns = size // P
                # xh: gathered head rows; xm: contiguous modifier rows.
                # Separate tiles so transposes of the gather half never wait
                # on the bwd DMA (tile-granular dependency tracking).
                xh = xh_pool.tile([P, ns, LD], cdt, tag="xh", name=f"xh_{gi}")
                xm = xm_pool.tile([P, ns, LD], cdt, tag="xm", name=f"xm_{gi}")
                for s in range(ns):
                    t = start // P + s
                    nc.gpsimd.indirect_dma_start(
                        out=xh[:, s, :],
                        out_offset=None,
                        in_=fwd[:],
                        in_offset=IndirectOffsetOnAxis(ap=hT_sb[:, t:t + 1], axis=0),
                    )
                nc.sync.dma_start(
                    xm[:],
                    bwd[start:start + size, :].rearrange("(s p) d -> p s d", p=P),
                )
                xg_tiles[gi] = (xh, xm)

            load_group(0)
            load_group(1)
            # w1 split per k-chunk so the first L1 matmul only waits on chunk 0
            w1_sb = [const_pool.tile([P, 2 * HID], cdt, tag=f"w1_{kc}",
                                     name=f"w1_{kc}")
                     for kc in range(4)]
            for kc in range(4):
                nc.sync.dma_start(w1_sb[kc][:], w1[kc * P:(kc + 1) * P, :])
            b1_sb = const_pool.tile([P, 8], f32)
            nc.sync.dma_start(b1_sb[:], b1[:])
            load_group(2)
            w2_sb = const_pool.tile([P, 8, HID2], cdt)
            nc.sync.dma_start(w2_sb[:], w2.rearrange("(kc p) j -> p kc j", p=P))
            b2_sb = const_pool.tile([P, 2], f32)
            nc.sync.dma_start(b2_sb[:], b2[:])
            load_group(3)
            w3_sb = const_pool.tile([P, 2, NREL], cdt)
            nc.sync.dma_start(w3_sb[:], w3.rearrange("(kc p) r -> p kc r", p=P))
            b3_sb = const_pool.tile([NREL, 1], f32)
            nc.sync.dma_start(b3_sb[:], b3[:])
            load_group(4)
            load_group(5)

            xT_tiles = [None] * len(GROUPS)

            def emit_transpose(gi):
                start, size = GROUPS[gi]
                xh, xm = xg_tiles[gi]
                xTs = []
                for kc in range(4):
                    src = xh if kc < 2 else xm
                    col = (kc % 2) * P
                    pt = pt_pool.tile([P, size], cdt, tag="pt", name=f"pt_{gi}_{kc}")
                    for s in range(size // P):
                        nc.tensor.transpose(
                            pt[:, s * P:(s + 1) * P],
                            src[:, s, col:col + P], ident[:])
                    xT = xT_pool.tile([P, size], cdt, tag="xT",
                                      name=f"xT_{gi}_{kc}")
                    nc.vector.tensor_copy(out=xT[:], in_=pt[:])
                    xTs.append(xT)
                xT_tiles[gi] = xTs

            emit_transpose(0)
            for gi, (start, size) in enumerate(GROUPS):
                xTs = xT_tiles[gi]
                # ---- layer 1: h = tanh(W1.T-chunks @ x + b1), 8 h-chunks ----
                h1s = []
                for hc in range(8):
                    ph = ph_pool.tile([P, size], f32, tag="ph",
                                      name=f"ph_{gi}_{hc}")
                    for kc in range(4):
                        nc.tensor.matmul(
                            out=ph[:],
                            lhsT=w1_sb[kc][:, hc * P:(hc + 1) * P],
                            rhs=xTs[kc][:],
                            start=(kc == 0),
                            stop=(kc == 3),
                        )
                    h1 = h1_pool.tile([P, size], cdt, tag="h1",
                                      name=f"h1_{gi}_{hc}")
                    nc.scalar.activation(
                        out=h1[:], in_=ph[:], func=Tanh,
                        bias=b1_sb[:, hc:hc + 1],
                    )
                    h1s.append(h1)

                # transpose the NEXT group now so its DVE copies finish
                # while this group's L2/L3 run on the PE
                if gi + 1 < len(GROUPS):
                    emit_transpose(gi + 1)

                # ---- layer 2: h2 = tanh(W2-chunks @ h + b2), 2 j-chunks ----
                h2s = []
                for jc in range(2):
                    pj = pj_pool.tile([P, size], f32, tag="pj",
                                      name=f"pj_{gi}_{jc}")
                    for kc in range(8):
                        nc.tensor.matmul(
                            out=pj[:],
                            lhsT=w2_sb[:, kc, jc * P:(jc + 1) * P],
                            rhs=h1s[kc][:],
                            start=(kc == 0),
                            stop=(kc == 7),
                        )
                    h2 = h2_pool.tile([P, size], cdt, tag="h2",
                                      name=f"h2_{gi}_{jc}")
                    nc.scalar.activation(
                        out=h2[:], in_=pj[:], func=Tanh,
                        bias=b2_sb[:, jc:jc + 1],
                    )
                    h2s.append(h2)

                # ---- layer 3: out = W3-chunks @ h2 + b3 ----
                po = po_pool.tile([NREL, size], f32, tag="pt", name=f"po_{gi}")
                for kc in range(2):
                    nc.tensor.matmul(
                        out=po[:],
                        lhsT=w3_sb[:, kc, :],
                        rhs=h2s[kc][:],
                        start=(kc == 0),
                        stop=(kc == 1),
                    )
                o = out_pool.tile([NREL, size], f32, tag="o", name=f"o_{gi}")
                nc.scalar.activation(
                    out=o[:], in_=po[:], func=Identity, bias=b3_sb[:, 0:1]
                )
                nc.sync.dma_start(outT[:, start:start + size], o[:])

    nc.finalize()
    return nc


def kernel(inputs, rhidLayerFOH, rhidLayerFOM, rcatBias, rhid2Layer, rhid2Bias,
           routLayer, routBias, heads):
    global LAST_RESULTS

    inputs = np.asarray(inputs, dtype=np.float32)
    heads = np.asarray(heads)

    if RUN_DT == "bf16":
        wdt = ml_dtypes.bfloat16
    else:
        wdt = np.float32

    fwd = np.ascontiguousarray(inputs[:, 0, :]).astype(wdt)      # [N, 256]
    bwd_full = inputs[:, 1, :]                                   # [N, 256]
    # mods for edge e is e+1; pad edge 16383 with mod 16383 (garbage, dropped)
    mods_pad = np.concatenate([np.arange(1, N_TOKENS), [N_TOKENS - 1]]).astype(np.int64)
    heads_pad = np.concatenate([heads.astype(np.int64), [0]]).astype(np.int32)

    w1 = np.ascontiguousarray(
        np.concatenate([np.asarray(rhidLayerFOH), np.asarray(rhidLayerFOM)], axis=1)
    ).astype(wdt)                                                # [512, 1024]
    w2 = np.ascontiguousarray(np.asarray(rhid2Layer)).astype(wdt)  # [1024, 256]
    w3 = np.ascontiguousarray(np.asarray(routLayer)).astype(wdt)   # [256, 64]
    b1 = np.ascontiguousarray(
        np.asarray(rcatBias, dtype=np.float32).reshape(8, P).T)    # [128, 8]
    b2 = np.ascontiguousarray(
        np.asarray(rhid2Bias, dtype=np.float32).reshape(2, P).T)   # [128, 2]
    b3 = np.ascontiguousarray(
        np.asarray(routBias, dtype=np.float32).reshape(1, NREL).T)  # [64, 1]

    in_maps = []
    for c in range(NCORES):
        sl = slice(c * EPC, (c + 1) * EPC)
        bwd_c = np.ascontiguousarray(bwd_full[mods_pad[sl]]).astype(wdt)  # [2048, 256]
        headsT_c = np.ascontiguousarray(
            heads_pad[sl].reshape(SUBTILES, P).T)                 # [128, 16]
        in_maps.append({
            "fwd": fwd, "bwd": bwd_c, "headsT": headsT_c,
            "w1": w1, "w2": w2, "w3": w3, "b1": b1, "b2": b2, "b3": b3,
        })

    if RUN_DT not in _CACHE:
        _CACHE[RUN_DT] = _build(RUN_DT)
    nc = _CACHE[RUN_DT]

    trace_dir = os.environ.get("KERNEL_TRACE_DIR") or None
    res = run_bass_kernel_spmd(nc, in_maps, list(range(NCORES)), tmpdir=trace_dir)
    LAST_RESULTS = res

    outT = np.concatenate([r["outT"] for r in res.results], axis=1)  # [64, 16384]
    return np.ascontiguousarray(outT.T[:E]).astype(np.float32)       # [16383, 64]
